# revision 3
# baseline (speedup 1.0000x reference)
"""Trainium2 Bass kernel for nn_CrossAggregator (GNN message passing).

Data-parallel over the node dimension N=10000 across 8 NeuronCores
(1250 nodes/core); all weights replicated. Per core the pipeline is:

  - gather exp(alpha[col]) via 9-way compare-select, softmax denom over S
  - weighted neighbor sum on DVE (per-partition-scalar multiply-accumulate)
  - PE transposes of self_vecs / neigh_sum tiles to feature-major
  - from_self / from_neighs: PSUM-accumulated matmuls, node-major output
  - attention head: [64,n] projections, @v, tanh/exp/softmax-of-2 gating
  - gated combine + relu, contiguous store
"""

import numpy as np
from contextlib import ExitStack

import concourse.bass as bass
import concourse.tile as tile
from concourse import bacc, mybir
from concourse.bass_utils import run_bass_kernel_spmd

N, S, D, DOUT, ATTEN, NATTR = 10000, 20, 512, 512, 64, 9
N_CORES = 8
NPC = N // N_CORES  # 1250 nodes per core
PT = 125            # nodes per tile (partition dim)
KC = D // 128       # 4 contraction chunks

f32 = mybir.dt.float32
i32 = mybir.dt.int32
FT = mybir.ActivationFunctionType
OP = mybir.AluOpType


def build_nc(npc=NPC, pt=PT, repeats=1):
    nt = npc // pt
    assert nt * pt == npc
    nc = bacc.Bacc("TRN2", target_bir_lowering=False, debug=False)

    sv = nc.dram_tensor("self_vecs", [npc, D], f32, kind="ExternalInput").ap()
    nv = nc.dram_tensor("neigh_vecs", [npc, S, D], f32, kind="ExternalInput").ap()
    nw = nc.dram_tensor("neigh_weight", [npc, S, 1], f32, kind="ExternalInput").ap()
    col = nc.dram_tensor("neigh_column", [npc, S, 1], i32, kind="ExternalInput").ap()
    wn = nc.dram_tensor("neigh_weights", [D, DOUT], f32, kind="ExternalInput").ap()
    ws = nc.dram_tensor("self_weights", [D, DOUT], f32, kind="ExternalInput").ap()
    al = nc.dram_tensor("alpha", [NATTR], f32, kind="ExternalInput").ap()
    sa = nc.dram_tensor("self_atten", [D, ATTEN], f32, kind="ExternalInput").ap()
    na = nc.dram_tensor("neigh_atten", [D, ATTEN], f32, kind="ExternalInput").ap()
    vv = nc.dram_tensor("v", [ATTEN, 1], f32, kind="ExternalInput").ap()
    out = nc.dram_tensor("out", [npc, DOUT], f32, kind="ExternalOutput").ap()

    ident_d = nc.inline_tensor(np.eye(128, dtype=np.float32), name="ident128")

    with tile.TileContext(nc) as tc, ExitStack() as ctx:
        consts = ctx.enter_context(tc.tile_pool(name="consts", bufs=1))
        prep = ctx.enter_context(tc.tile_pool(name="prep", bufs=1))
        nv_pool = ctx.enter_context(tc.tile_pool(name="nvp", bufs=2))
        x_pool = ctx.enter_context(tc.tile_pool(name="xp", bufs=2))
        acc_pool = ctx.enter_context(tc.tile_pool(name="accp", bufs=2))
        xT_pool = ctx.enter_context(tc.tile_pool(name="xTp", bufs=2))
        nsT_pool = ctx.enter_context(tc.tile_pool(name="nsTp", bufs=2))
        ai_sb_pool = ctx.enter_context(tc.tile_pool(name="aisb", bufs=2))
        sm_pool = ctx.enter_context(tc.tile_pool(name="smalls", bufs=2))
        big_pool = ctx.enter_context(tc.tile_pool(name="bigs", bufs=2))

        ps_pre = ctx.enter_context(tc.tile_pool(name="ps_pre", bufs=1, space="PSUM"))
        ps_tx = ctx.enter_context(tc.tile_pool(name="ps_tx", bufs=1, space="PSUM"))
        ps_tn = ctx.enter_context(tc.tile_pool(name="ps_tn", bufs=1, space="PSUM"))
        ps_fs = ctx.enter_context(tc.tile_pool(name="ps_fs", bufs=1, space="PSUM"))
        ps_fn = ctx.enter_context(tc.tile_pool(name="ps_fn", bufs=1, space="PSUM"))
        ps_ai = ctx.enter_context(tc.tile_pool(name="ps_ai", bufs=1, space="PSUM"))
        ps_u = ctx.enter_context(tc.tile_pool(name="ps_u", bufs=1, space="PSUM"))

        # ---- constants ----
        ident = consts.tile([128, 128], f32, tag="ident")
        nc.gpsimd.dma_start(ident[:], ident_d.ap())

        ws_sb = consts.tile([128, KC * DOUT], f32, tag="ws")
        wn_sb = consts.tile([128, KC * DOUT], f32, tag="wn")
        sa_sb = consts.tile([128, KC * ATTEN], f32, tag="sa")
        na_sb = consts.tile([128, KC * ATTEN], f32, tag="na")
        for c in range(KC):
            nc.gpsimd.dma_start(ws_sb[:, c * DOUT:(c + 1) * DOUT],
                                ws[c * 128:(c + 1) * 128, :])
            nc.gpsimd.dma_start(wn_sb[:, c * DOUT:(c + 1) * DOUT],
                                wn[c * 128:(c + 1) * 128, :])
            nc.gpsimd.dma_start(sa_sb[:, c * ATTEN:(c + 1) * ATTEN],
                                sa[c * 128:(c + 1) * 128, :])
            nc.gpsimd.dma_start(na_sb[:, c * ATTEN:(c + 1) * ATTEN],
                                na[c * 128:(c + 1) * 128, :])

        v_sb = consts.tile([ATTEN, 1], f32, tag="v")
        v2_sb = consts.tile([ATTEN, 1], f32, tag="v2")
        nc.gpsimd.dma_start(v_sb[:], vv[:, :])
        nc.scalar.mul(v2_sb[:], v_sb[:], 2.0)

        # exp(alpha), broadcast to all partitions via K=1 matmul with ones
        al_sb = consts.tile([1, NATTR], f32, tag="al")
        ea1 = consts.tile([1, NATTR], f32, tag="ea1")
        ones = consts.tile([1, 128], f32, tag="ones")
        ea_bc = consts.tile([128, NATTR], f32, tag="eabc")
        nc.gpsimd.dma_start(al_sb[:], al[None, :])
        nc.scalar.activation(ea1[:], al_sb[:], FT.Exp)
        nc.vector.memset(ones[:], 1.0)
        ea_ps = ps_pre.tile([128, NATTR], f32, tag="eaps")
        nc.tensor.matmul(ea_ps[:], lhsT=ones[:], rhs=ea1[:], start=True, stop=True)
        nc.scalar.copy(ea_bc[:], ea_ps[:])

        # ---- per-core softmax-weight precompute (all nt tiles at once) ----
        col_all = prep.tile([pt, nt * S], i32, tag="col")
        colf = prep.tile([pt, nt * S], f32, tag="colf")
        nw_all = prep.tile([pt, nt * S], f32, tag="nw")
        mask = prep.tile([pt, nt * S], f32, tag="mask")
        e_all = prep.tile([pt, nt * S], f32, tag="e")
        z_all = prep.tile([pt, nt], f32, tag="z")
        rz_all = prep.tile([pt, nt], f32, tag="rz")

        nc.gpsimd.dma_start(col_all[:].rearrange("p (t s) -> p t s", t=nt),
                            col.rearrange("(t p) s o -> p t (s o)", p=pt))
        nc.gpsimd.dma_start(nw_all[:].rearrange("p (t s) -> p t s", t=nt),
                            nw.rearrange("(t p) s o -> p t (s o)", p=pt))
        nc.vector.tensor_copy(colf[:], col_all[:])
        for k in range(NATTR):
            nc.vector.tensor_scalar(mask[:], colf[:], float(k), None, OP.is_equal)
            if k == 0:
                nc.vector.tensor_scalar_mul(e_all[:], mask[:], ea_bc[:pt, 0:1])
            else:
                nc.vector.scalar_tensor_tensor(
                    e_all[:], mask[:], ea_bc[:pt, k:k + 1], e_all[:],
                    op0=OP.mult, op1=OP.add)
        nc.vector.reduce_sum(
            z_all[:], e_all[:].rearrange("p (t s) -> p t s", t=nt),
            axis=mybir.AxisListType.X)
        # fold neigh_weight in: w = exp(a)*nw ; 1/Z applied after the S-sum
        nc.vector.tensor_mul(e_all[:], e_all[:], nw_all[:])
        nc.vector.reciprocal(rz_all[:], z_all[:])

        def tile_body(t):
            nvt = nv_pool.tile([pt, S * D], f32, tag="nvt")
            nc.sync.dma_start(
                nvt[:], nv[t * pt:(t + 1) * pt].rearrange("p s d -> p (s d)"))
            xt = x_pool.tile([pt, D], f32, tag="xt")
            nc.sync.dma_start(xt[:], sv[t * pt:(t + 1) * pt, :])

            # weighted neighbor sum (node-major), 1/Z folded at the end
            acc = acc_pool.tile([pt, D], f32, tag="acc")
            for s in range(S):
                w_ap = e_all[:, t * S + s:t * S + s + 1]
                if s == 0:
                    nc.vector.tensor_scalar_mul(acc[:], nvt[:, 0:D], w_ap)
                else:
                    nc.vector.scalar_tensor_tensor(
                        acc[:], nvt[:, s * D:(s + 1) * D], w_ap, acc[:],
                        op0=OP.mult, op1=OP.add)
            nc.vector.tensor_scalar_mul(acc[:], acc[:], rz_all[:, t:t + 1])

            # transpose x and neigh_sum to feature-major [128d, pt]
            xT_ps = ps_tx.tile([128, KC * pt], f32, tag="xTps")
            nsT_ps = ps_tn.tile([128, KC * pt], f32, tag="nsTps")
            for c in range(KC):
                nc.tensor.transpose(xT_ps[:, c * pt:(c + 1) * pt],
                                    xt[:, c * 128:(c + 1) * 128],
                                    ident[:pt, :pt])
            for c in range(KC):
                nc.tensor.transpose(nsT_ps[:, c * pt:(c + 1) * pt],
                                    acc[:, c * 128:(c + 1) * 128],
                                    ident[:pt, :pt])
            xT = xT_pool.tile([128, KC * pt], f32, tag="xT")
            nsT = nsT_pool.tile([128, KC * pt], f32, tag="nsT")
            nc.scalar.copy(xT[:], xT_ps[:])
            nc.scalar.copy(nsT[:], nsT_ps[:])

            # from_self / from_neighs: [pt, DOUT] node-major in PSUM
            fs_ps = ps_fs.tile([pt, DOUT], f32, tag="fsps")
            fn_ps = ps_fn.tile([pt, DOUT], f32, tag="fnps")
            for c in range(KC):
                nc.tensor.matmul(fs_ps[:], lhsT=xT[:, c * pt:(c + 1) * pt],
                                 rhs=ws_sb[:, c * DOUT:(c + 1) * DOUT],
                                 start=(c == 0), stop=(c == KC - 1))
            for c in range(KC):
                nc.tensor.matmul(fn_ps[:], lhsT=nsT[:, c * pt:(c + 1) * pt],
                                 rhs=wn_sb[:, c * DOUT:(c + 1) * DOUT],
                                 start=(c == 0), stop=(c == KC - 1))

            # attention projections, feature-major [ATTEN, pt]
            ai_ps = ps_ai.tile([ATTEN, 2 * pt], f32, tag="aips")
            for c in range(KC):
                nc.tensor.matmul(ai_ps[:, 0:pt],
                                 lhsT=sa_sb[:, c * ATTEN:(c + 1) * ATTEN],
                                 rhs=xT[:, c * pt:(c + 1) * pt],
                                 start=(c == 0), stop=(c == KC - 1))
            for c in range(KC):
                nc.tensor.matmul(ai_ps[:, pt:2 * pt],
                                 lhsT=na_sb[:, c * ATTEN:(c + 1) * ATTEN],
                                 rhs=nsT[:, c * pt:(c + 1) * pt],
                                 start=(c == 0), stop=(c == KC - 1))
            ai_s = ai_sb_pool.tile([ATTEN, pt], f32, tag="ais")
            ai_ns = ai_sb_pool.tile([ATTEN, pt], f32, tag="ains")
            nc.scalar.copy(ai_s[:], ai_ps[:, 0:pt])
            nc.vector.tensor_add(ai_ns[:], ai_s[:], ai_ps[:, pt:2 * pt])

            # u_self = (2*self_ai) @ v, u_neigh = (neigh_ai+self_ai) @ v
            u_ps = ps_u.tile([pt, 2], f32, tag="ups")
            nc.tensor.matmul(u_ps[:, 0:1], lhsT=ai_s[:], rhs=v2_sb[:],
                             start=True, stop=True)
            nc.tensor.matmul(u_ps[:, 1:2], lhsT=ai_ns[:], rhs=v_sb[:],
                             start=True, stop=True)

            ts_t = sm_pool.tile([pt, 1], f32, tag="ts")
            es_t = sm_pool.tile([pt, 1], f32, tag="es")
            tn_t = sm_pool.tile([pt, 1], f32, tag="tn")
            en_t = sm_pool.tile([pt, 1], f32, tag="en")
            den = sm_pool.tile([pt, 1], f32, tag="den")
            rde = sm_pool.tile([pt, 1], f32, tag="rde")
            rs_t = sm_pool.tile([pt, 1], f32, tag="rs")
            rn_t = sm_pool.tile([pt, 1], f32, tag="rn")
            nc.scalar.activation(ts_t[:], u_ps[:, 0:1], FT.Tanh)
            nc.scalar.activation(tn_t[:], u_ps[:, 1:2], FT.Tanh)
            nc.scalar.activation(es_t[:], ts_t[:], FT.Exp)
            nc.scalar.activation(en_t[:], tn_t[:], FT.Exp)
            nc.vector.tensor_add(den[:], es_t[:], en_t[:])
            nc.vector.reciprocal(rde[:], den[:])
            nc.vector.tensor_mul(rs_t[:], es_t[:], rde[:])
            nc.vector.tensor_mul(rn_t[:], en_t[:], rde[:])

            # gated combine + relu, store
            tmp = big_pool.tile([pt, DOUT], f32, tag="tmp")
            comb = big_pool.tile([pt, DOUT], f32, tag="comb")
            outsb = big_pool.tile([pt, DOUT], f32, tag="outsb")
            nc.vector.tensor_scalar_mul(tmp[:], fs_ps[:], rs_t[:, 0:1])
            nc.vector.scalar_tensor_tensor(
                comb[:], fn_ps[:], rn_t[:, 0:1], tmp[:], op0=OP.mult, op1=OP.add)
            nc.scalar.activation(outsb[:], comb[:], FT.Relu)
            nc.scalar.dma_start(out[t * pt:(t + 1) * pt, :], outsb[:])

        if repeats > 1:
            with tc.For_i(0, repeats, 1) as _i:
                for t in range(nt):
                    tile_body(t)
        else:
            for t in range(nt):
                tile_body(t)

    nc.compile()
    return nc


_nc_cache = {}


def get_nc(npc=NPC, pt=PT, repeats=1):
    key = (npc, pt, repeats)
    if key not in _nc_cache:
        _nc_cache[key] = build_nc(*key)
    return _nc_cache[key]


def make_in_maps(inputs, npc=NPC):
    shared = {k: np.ascontiguousarray(np.asarray(inputs[k]))
              for k in ("neigh_weights", "self_weights", "alpha",
                        "self_atten", "neigh_atten", "v")}
    in_maps = []
    for c in range(N_CORES):
        sl = slice(c * npc, (c + 1) * npc)
        m = dict(shared)
        for k in ("self_vecs", "neigh_vecs", "neigh_weight", "neigh_column"):
            m[k] = np.ascontiguousarray(np.asarray(inputs[k])[sl])
        in_maps.append(m)
    return in_maps


def kernel(**inputs):
    nc = get_nc()
    in_maps = make_in_maps(inputs)
    res = run_bass_kernel_spmd(nc, in_maps, list(range(N_CORES)))
    return np.concatenate([res.results[c]["out"] for c in range(N_CORES)], axis=0)
